# revision 8
# baseline (speedup 1.0000x reference)
"""Trainium2 Bass kernel for nn_DeltaResBlock.

Math (per batch element b):
    x = bf16(x0)                       # [D, V]
    pool_d = mean_v x                  # [D]
    pool_v = mean_d x                  # [V]
    k = MLP_k(pool_d); k /= (||k|| + 1e-6)
    v = MLP_v(pool_v)
    beta = 2*sigmoid(MLP_b(pool_d))
    proj = k @ x                       # [V]
    out = relu(x + beta * outer(k, v - proj))

Sharding: data-parallel over batch B=1024 across 8 cores (128 each),
params replicated. Per core the batch is processed in chunks of NB.

On-chip layout: x lives as [128(d%128), 4(d//128), 256(v)] bf16 tiles per
batch element. All cross-partition reductions (pool_v, proj, ||k||) and the
MLPs run on the tensor engine in "transposed" [feature-on-partition, batch]
layouts so every PSUM output starts at partition 0.
"""

import sys
from contextlib import ExitStack

sys.path.insert(0, "/opt/trn_rl_repo")

import numpy as np
import ml_dtypes

import concourse.bass as bass
import concourse.bacc as bacc
import concourse.mybir as mybir
import concourse.tile as tile
from concourse.bass_utils import run_bass_kernel_spmd
from concourse.masks import make_identity

bf16 = ml_dtypes.bfloat16
F32 = mybir.dt.float32
BF = mybir.dt.bfloat16
ALU = mybir.AluOpType
ACTF = mybir.ActivationFunctionType

B, D, V = 1024, 512, 256
NODE, GATE = 1024, 128
N_CORES = 8
EPS_K = 1e-6
# BatchNorm inference scale; the reference casts it to bf16 (== exactly 1.0).
S_BN = float(np.float32(bf16(1.0 / np.sqrt(1.0 + 1e-5))))

DT = D // 128     # 4  d-tiles
VT = V // 128     # 2  v-half tiles
NT = NODE // 128  # 8 node tiles


def build_nc(b_core=128, nb=32):
    nch = b_core // nb
    assert nch * nb == b_core
    nc = bacc.Bacc("TRN2", target_bir_lowering=False, debug=False, num_devices=N_CORES)

    x0 = nc.dram_tensor("x0", [b_core, D, V], F32, kind="ExternalInput")
    wk1 = nc.dram_tensor("W_k1", [D, NODE], F32, kind="ExternalInput")
    bk1 = nc.dram_tensor("b_k1", [NODE], F32, kind="ExternalInput")
    wk2 = nc.dram_tensor("W_k2", [NODE, D], F32, kind="ExternalInput")
    bk2 = nc.dram_tensor("b_k2", [D], F32, kind="ExternalInput")
    wv1 = nc.dram_tensor("W_v1", [V, NODE], F32, kind="ExternalInput")
    bv1 = nc.dram_tensor("b_v1", [NODE], F32, kind="ExternalInput")
    wv2 = nc.dram_tensor("W_v2", [NODE, V], F32, kind="ExternalInput")
    bv2 = nc.dram_tensor("b_v2", [V], F32, kind="ExternalInput")
    wb1 = nc.dram_tensor("W_b1", [D, GATE], F32, kind="ExternalInput")
    bb1 = nc.dram_tensor("b_b1", [GATE], F32, kind="ExternalInput")
    wb2 = nc.dram_tensor("W_b2", [GATE, 1], F32, kind="ExternalInput")
    bb2 = nc.dram_tensor("b_b2", [1], F32, kind="ExternalInput")
    out = nc.dram_tensor("out", [b_core, D, V], BF, kind="ExternalOutput")
    wbounce = nc.dram_tensor("w_bounce", [nch, nb, V], BF)

    with tile.TileContext(nc) as tc, ExitStack() as ctx:
        consts = ctx.enter_context(tc.tile_pool(name="consts", bufs=1))

        # ---- replicated params: load fp32, cast to bf16 on chip ----
        wk1b = consts.tile([128, DT, NODE], BF)
        wk2b = consts.tile([128, NT, D], BF)
        wv1b = consts.tile([128, VT, NODE], BF)
        wv2b = consts.tile([128, NT, V], BF)
        wb1b = consts.tile([128, DT, GATE], BF)
        wb2b = consts.tile([128, 1], BF)
        with tc.tile_pool(name="wstage", bufs=1) as wstage:
            for dst, src in (
                (wk1b, wk1.rearrange("(t p) n -> p t n", p=128)),
                (wk2b, wk2.rearrange("(t p) n -> p t n", p=128)),
                (wv1b, wv1.rearrange("(t p) n -> p t n", p=128)),
                (wv2b, wv2.rearrange("(t p) n -> p t n", p=128)),
                (wb1b, wb1.rearrange("(t p) n -> p t n", p=128)),
                (wb2b, wb2[:, :]),
            ):
                st = wstage.tile(list(dst.shape), F32, tag="wstage")
                nc.sync.dma_start(out=st, in_=src)
                nc.vector.tensor_copy(dst, st)

            # biases: quantize to bf16 (reference casts params to bf16), keep
            # fp32 copies for ACT bias operands; fold the BN scale into b_k1/b_v1.
            def bias_tile(name, src_ap, cols, scale=None):
                stg = wstage.tile([128, cols], F32, tag="bstage")
                nc.sync.dma_start(out=stg, in_=src_ap)
                q = wstage.tile([128, cols], BF, tag="bq")
                nc.vector.tensor_copy(q, stg)
                dst = consts.tile([128, cols], F32, tag=name)
                if scale is not None and scale != 1.0:
                    nc.vector.tensor_scalar_mul(dst, q, scale)
                else:
                    nc.vector.tensor_copy(dst, q)
                return dst

            bk1s = bias_tile("bk1s", bk1.rearrange("(t p) -> p t", p=128), NT, S_BN)
            bv1s = bias_tile("bv1s", bv1.rearrange("(t p) -> p t", p=128), NT, S_BN)
            bk2s = bias_tile("bk2s", bk2.rearrange("(t p) -> p t", p=128), DT)
            bb1s = bias_tile("bb1s", bb1.rearrange("(p x) -> p x", x=1), 1)

            bb2s = consts.tile([1, 1], F32)
            stg2 = wstage.tile([1, 1], F32, tag="bb2stage")
            nc.sync.dma_start(out=stg2, in_=bb2.rearrange("(a x) -> a x", a=1))
            q2 = wstage.tile([1, 1], BF, tag="bb2q")
            nc.vector.tensor_copy(q2, stg2)
            nc.vector.tensor_copy(bb2s, q2)

            bv2r = consts.tile([1, V], BF)
            stv = wstage.tile([1, V], F32, tag="bv2stage")
            nc.sync.dma_start(out=stv, in_=bv2.rearrange("(a v) -> a v", a=1))
            nc.vector.tensor_copy(bv2r, stv)

        ones_row = consts.tile([1, nb], BF)
        nc.vector.memset(ones_row, 1.0)
        ones_col = consts.tile([128, 1], BF)
        nc.vector.memset(ones_col, 1.0 / D)  # folds the pool_v mean
        ones_f32 = consts.tile([128, 1], F32)
        nc.vector.memset(ones_f32, 1.0)
        ident = consts.tile([128, 128], BF)
        make_identity(nc, ident)

        # ---- per-chunk pools ----
        xf_pool = ctx.enter_context(tc.tile_pool(name="xf", bufs=3))
        xbf_pool = ctx.enter_context(
            tc.tile_pool(name="xbf", bufs=min(b_core, nb + 12))
        )
        small = ctx.enter_context(tc.tile_pool(name="small", bufs=2))
        wbc_pool = ctx.enter_context(tc.tile_pool(name="wbc", bufs=6))
        out_pool = ctx.enter_context(tc.tile_pool(name="outb", bufs=6))
        tmp_pool = ctx.enter_context(tc.tile_pool(name="tmpb", bufs=4))
        psum = ctx.enter_context(tc.tile_pool(name="psum", bufs=1, space="PSUM"))

        for c in range(nch):
            b0 = c * nb

            # --- phase A: load, cast, pool_d (DVE), pool_v (PE) ---
            pd_f = small.tile([128, DT, nb], F32, tag="pd_f")
            ppv = psum.tile([128, VT, nb], F32, tag="ppv")
            xbs = []
            for i in range(nb):
                xf = xf_pool.tile([128, DT, V], F32, tag="xf")
                nc.sync.dma_start(
                    out=xf, in_=x0[b0 + i].rearrange("(t p) v -> p t v", p=128)
                )
                xb = xbf_pool.tile([128, DT, V], BF, tag="xb")
                xbs.append(xb)
                for t in range(DT):
                    if t < 2:
                        nc.vector.tensor_scalar(
                            out=xb[:, t, :], in0=xf[:, t, :],
                            scalar1=1.0, scalar2=0.0, op0=ALU.mult, op1=ALU.add,
                            accum_out=pd_f[:, t, i : i + 1],
                        )
                    else:
                        nc.scalar.activation(
                            out=xb[:, t, :], in_=xf[:, t, :], func=ACTF.Copy,
                            accum_out=pd_f[:, t, i : i + 1],
                        )
                for h in range(VT):
                    for t in range(DT):
                        nc.tensor.matmul(
                            ppv[:, h, i : i + 1],
                            lhsT=xb[:, t, h * 128 : (h + 1) * 128],
                            rhs=ones_col,
                            start=(t == 0),
                            stop=(t == DT - 1),
                        )

            pd_bf = small.tile([128, DT, nb], BF, tag="pd_bf")
            nc.vector.tensor_scalar_mul(pd_bf, pd_f, 1.0 / V)
            pvT = small.tile([128, VT, nb], BF, tag="pvT")
            nc.scalar.copy(out=pvT, in_=ppv)

            # --- k branch: h1 = relu((pool_d @ Wk1 + b1) * s_bn), node-on-partition ---
            ph1 = psum.tile([128, NT, nb], F32, tag="ph1")
            for mt in range(NT):
                for t in range(DT):
                    nc.tensor.matmul(
                        ph1[:, mt, :],
                        lhsT=wk1b[:, t, mt * 128 : (mt + 1) * 128],
                        rhs=pd_bf[:, t, :],
                        start=(t == 0),
                        stop=(t == DT - 1),
                    )
            h1r = small.tile([128, NT, nb], BF, tag="h1r")
            for mt in range(NT):
                nc.scalar.activation(
                    out=h1r[:, mt, :], in_=ph1[:, mt, :], func=ACTF.Relu,
                    bias=bk1s[:, mt : mt + 1], scale=S_BN,
                )

            # k = h1 @ Wk2 + b2, d-on-partition; pkt[:, DT, :] doubles as the
            # beta-branch hidden psum.
            pkt = psum.tile([128, DT + 1, nb], F32, tag="pkt")
            for dt in range(DT):
                for nt in range(NT):
                    nc.tensor.matmul(
                        pkt[:, dt, :],
                        lhsT=wk2b[:, nt, dt * 128 : (dt + 1) * 128],
                        rhs=h1r[:, nt, :],
                        start=(nt == 0),
                        stop=(nt == NT - 1),
                    )
            kc = small.tile([128, DT, nb], BF, tag="kc")
            for dt in range(DT):
                nc.scalar.activation(
                    out=kc[:, dt, :], in_=pkt[:, dt, :], func=ACTF.Identity,
                    bias=bk2s[:, dt : dt + 1], scale=1.0,
                )

            # ||k||: square on DVE, partition-sum on PE
            ksq = small.tile([128, DT, nb], F32, tag="ksq")
            nc.vector.tensor_mul(ksq, kc, kc)
            pnrm = psum.tile([1, 2, nb], F32, tag="pnrm")
            for dt in range(DT):
                nc.tensor.matmul(
                    pnrm[0:1, 0, :], lhsT=ones_f32, rhs=ksq[:, dt, :],
                    start=(dt == 0), stop=(dt == DT - 1),
                )
            nrm = small.tile([1, nb], F32, tag="nrm")
            nc.scalar.activation(out=nrm, in_=pnrm[0:1, 0, :], func=ACTF.Sqrt)
            nc.vector.tensor_scalar_add(nrm, nrm, EPS_K)
            rrow = small.tile([1, nb], F32, tag="rrow")
            nc.vector.reciprocal(rrow, nrm)
            rbc = small.tile([128, nb], F32, tag="rbc")
            nc.gpsimd.partition_broadcast(rbc, rrow)
            kn = small.tile([128, DT, nb], BF, tag="kn")
            nc.vector.tensor_mul(kn, kc, rbc[:, None, :].broadcast_to([128, DT, nb]))
            kn_f = small.tile([128, DT, nb], F32, tag="kn_f")
            nc.vector.tensor_copy(kn_f, kn)

            # --- v branch ---
            phv = psum.tile([128, NT, nb], F32, tag="phv")
            for mt in range(NT):
                for vt in range(VT):
                    nc.tensor.matmul(
                        phv[:, mt, :],
                        lhsT=wv1b[:, vt, mt * 128 : (mt + 1) * 128],
                        rhs=pvT[:, vt, :],
                        start=(vt == 0),
                        stop=(vt == VT - 1),
                    )
            hvr = small.tile([128, NT, nb], BF, tag="hvr")
            for mt in range(NT):
                nc.scalar.activation(
                    out=hvr[:, mt, :], in_=phv[:, mt, :], func=ACTF.Relu,
                    bias=bv1s[:, mt : mt + 1], scale=S_BN,
                )
            pvv = psum.tile([128, VT, nb], F32, tag="pvv")
            for h in range(VT):
                for nt in range(NT):
                    nc.tensor.matmul(
                        pvv[:, h, :],
                        lhsT=wv2b[:, nt, h * 128 : (h + 1) * 128],
                        rhs=hvr[:, nt, :],
                        start=(nt == 0),
                        stop=False,
                    )
                # rank-1 bias add: v += b_v2 ⊗ 1
                nc.tensor.matmul(
                    pvv[:, h, :],
                    lhsT=bv2r[0:1, h * 128 : (h + 1) * 128],
                    rhs=ones_row,
                    start=False,
                    stop=True,
                )

            # --- beta branch ---
            for t in range(DT):
                nc.tensor.matmul(
                    pkt[:, DT, :], lhsT=wb1b[:, t, :], rhs=pd_bf[:, t, :],
                    start=(t == 0), stop=(t == DT - 1),
                )
            bh = small.tile([128, nb], BF, tag="bh")
            nc.scalar.activation(
                out=bh, in_=pkt[:, DT, :], func=ACTF.Tanh,
                bias=bb1s[:, 0:1], scale=1.0,
            )
            nc.tensor.matmul(pnrm[0:1, 1, :], lhsT=wb2b, rhs=bh, start=True, stop=True)
            sg = small.tile([1, nb], F32, tag="sg")
            nc.scalar.activation(
                out=sg, in_=pnrm[0:1, 1, :], func=ACTF.Sigmoid, bias=bb2s[0:1, 0:1],
            )
            nc.vector.tensor_scalar_mul(sg, sg, 2.0)
            bbc = small.tile([128, nb], F32, tag="bbc")
            nc.gpsimd.partition_broadcast(bbc, sg)

            # --- proj = k·x (per b, v-half-on-partition) ---
            pproj = psum.tile([128, VT, nb], F32, tag="pproj")
            for i in range(nb):
                for h in range(VT):
                    for t in range(DT):
                        nc.tensor.matmul(
                            pproj[:, h, i : i + 1],
                            lhsT=xbs[i][:, t, h * 128 : (h + 1) * 128],
                            rhs=kn[:, t, i : i + 1],
                            start=(t == 0),
                            stop=(t == DT - 1),
                        )

            # --- w = beta * (v - proj), then transpose to per-b rows ---
            v_sb = small.tile([128, VT, nb], BF, tag="v_sb")
            nc.scalar.copy(out=v_sb, in_=pvv)
            wvf = small.tile([128, VT, nb], F32, tag="wvf")
            nc.vector.scalar_tensor_tensor(
                out=wvf, in0=pproj, scalar=-1.0, in1=v_sb, op0=ALU.mult, op1=ALU.add,
            )
            wvT = small.tile([128, VT, nb], BF, tag="wvT")
            nc.vector.tensor_mul(wvT, wvf, bbc[:, None, :].broadcast_to([128, VT, nb]))
            ptr = psum.tile([nb, VT, 128], BF, tag="ptr")
            for h in range(VT):
                nc.tensor.transpose(ptr[:, h, :], wvT[:, h, :], ident)
            wrow = small.tile([nb, VT, 128], BF, tag="wrow")
            nc.vector.tensor_copy(wrow, ptr)
            nc.sync.dma_start(
                out=wbounce[c].rearrange("b (h p) -> b h p", h=VT), in_=wrow
            )

            # --- update: out = relu(x + (w ⊗ k)) ---
            for i in range(nb):
                wbc = wbc_pool.tile([128, V], BF, tag="wbc")
                src = wbounce[c, i, :]
                nc.sync.dma_start(
                    out=wbc,
                    in_=bass.AP(tensor=src.tensor, offset=src.offset,
                                ap=[[0, 128]] + src.ap),
                )
                ob = out_pool.tile([128, DT, V], BF, tag="ob")
                tmp = tmp_pool.tile([128, DT, V], BF, tag="tmp")
                for t in range(DT):
                    nc.vector.tensor_scalar_mul(
                        tmp[:, t, :], wbc, kn_f[:, t, i : i + 1]
                    )
                nc.vector.tensor_add(ob, tmp, xbs[i])
                nc.scalar.activation(out=ob, in_=ob, func=ACTF.Relu)
                nc.sync.dma_start(
                    out=out[b0 + i].rearrange("(t p) v -> p t v", p=128), in_=ob
                )

    nc.compile()
    return nc


_NC_CACHE = {}


def _get_nc(b_core=128, nb=32):
    key = (b_core, nb)
    if key not in _NC_CACHE:
        _NC_CACHE[key] = build_nc(b_core, nb)
    return _NC_CACHE[key]


def _run(inputs, trace=False, **kwargs):
    x0 = np.ascontiguousarray(np.asarray(inputs["x0"], dtype=np.float32))
    assert x0.shape == (B, D, V)
    params = {
        k: np.ascontiguousarray(np.asarray(inputs[k], dtype=np.float32))
        for k in (
            "W_k1", "b_k1", "W_k2", "b_k2", "W_v1", "b_v1", "W_v2", "b_v2",
            "W_b1", "b_b1", "W_b2", "b_b2",
        )
    }
    params["b_b2"] = params["b_b2"].reshape(1)
    params["W_b2"] = params["W_b2"].reshape(GATE, 1)

    b_core = B // N_CORES
    nc = _get_nc(b_core=b_core, nb=32)
    in_maps = [
        {"x0": x0[c * b_core : (c + 1) * b_core], **params} for c in range(N_CORES)
    ]
    res = run_bass_kernel_spmd(
        nc, in_maps, core_ids=list(range(N_CORES)), trace=trace, **kwargs
    )
    out = np.concatenate([res.results[c]["out"] for c in range(N_CORES)], axis=0)
    return out.astype(bf16), res


def kernel(**inputs) -> np.ndarray:
    out, _ = _run(inputs, trace=False)
    return out
